# revision 1
# baseline (speedup 1.0000x reference)
"""CombinedSparsity (spatial max-pool + lifetime top-k + max-unpool) on 8 TRN2 cores.

Shard the 128 channels across 8 cores (16 each). Per (b, c) map the output is
all zeros except (possibly) one element: the map's max, written back at its
argmax position, kept only if that max is among the top-6 over the batch for
its channel. Output buffers are donated zero-filled, so each core only writes
the surviving elements.

Per channel (streamed one 2MB map-block at a time):
  - the streaming reduce keeps 16 chunk maxima (same DVE cost as a flat max),
  - a small FIND_INDEX8 over the chunk maxima locates the (first) chunk that
    contains the map's max — the exact argmax chunk of every (b, c) map
    without a second full pass.
Per unit of 4 channels:
  - top-8 over the batch per channel via InstMax/InstMaxIndex on the PE-
    transposed pooled matrix (progressive duplicate handling keeps exact-tie
    batches identical to jax.lax.top_k),
  - a one-hot survivor matrix is built on the Scalar engine from the PE-
    broadcast top-6 batch indices; per-channel PE matmuls compact each
    survivor's [chunk row | max] pair; one more PE transpose yields [24, 2],
  - ONE indirect DMA gathers the 24 surviving 1KB chunks (24 descriptors --
    indirect-DMA descriptor generation steals bandwidth from DMA engine E79,
    so survivors are compacted BEFORE gathering, not after),
  - three Scalar activation passes mask each chunk to {max at argmax, 0},
  - one indirect DMA scatters the 24 masked rows into the output.
Vector only runs the streaming reduces plus tiny per-channel/unit ops, so it
never stalls on DMA round trips; all glue runs on GpSimd/Scalar/PE. Stream
DMA triggers rotate across Sync/Scalar/Tensor queues so startup descriptor
generation pipelines.
"""
import numpy as np

import concourse.bass as bass
import concourse.bacc as bacc
import concourse.tile as tile
from concourse import mybir
from concourse.bass_utils import run_bass_kernel_spmd
from concourse.masks import make_identity

B = 128
C_FULL = 128
H = 64
W = 64
HW = H * W
N_CORES = 8
CSH = C_FULL // N_CORES      # channels per core
K = 6                        # lifetime top-k
NCHUNK = 16                  # chunk maxima kept per map
CHW = HW // NCHUNK           # elements per chunk
F32 = mybir.dt.float32
I32 = mybir.dt.int32
U32 = mybir.dt.uint32
UNITS = [(0, 4), (4, 8), (8, 12), (12, 16)]

_nc_cache = None


def _build():
    global _nc_cache
    if _nc_cache is not None:
        return _nc_cache

    nc = bacc.Bacc("TRN2", target_bir_lowering=False, debug=False)
    x = nc.dram_tensor("x", [B, CSH, HW], F32, kind="ExternalInput")
    y = nc.dram_tensor("y", [B, CSH, HW], F32, kind="ExternalOutput")
    x_rows = x.rearrange("b c (k j) -> (b c k) j", j=CHW)
    y_rows = y.rearrange("b c (k j) -> (b c k) j", j=CHW)

    with tile.TileContext(nc) as tc:
        with (
            tc.tile_pool(name="const", bufs=1) as cp,
            tc.tile_pool(name="gxp", bufs=5) as gxp,
            tc.tile_pool(name="pmp", bufs=4) as pmp,
            tc.tile_pool(name="small", bufs=1) as sp,
            tc.tile_pool(name="ps", bufs=1, space="PSUM") as pp,
        ):
            ident0 = cp.tile([B, B], F32)
            make_identity(nc, ident0[:])
            ident = cp.tile([B, B], F32)
            nc.vector.tensor_copy(out=ident[:], in_=ident0[:])
            ones_row = cp.tile([1, B], F32)
            nc.gpsimd.memset(ones_row[:], 1.0)
            iota_b_i = cp.tile([B, 1], I32)
            nc.gpsimd.iota(iota_b_i[:], pattern=[[1, 1]], base=0,
                           channel_multiplier=1)
            iota_b = cp.tile([B, 1], F32)
            nc.gpsimd.tensor_copy(out=iota_b[:], in_=iota_b_i[:])
            # b*CSH*NCHUNK: row of (b, c=0, chunk=0) in x_rows/y_rows
            brow = cp.tile([B, 1], F32)
            nc.gpsimd.tensor_scalar(
                out=brow[:], in0=iota_b[:], scalar1=float(CSH * NCHUNK),
                scalar2=None, op0=mybir.AluOpType.mult,
            )

            pooled = sp.tile([B, CSH], F32, name="pooled")
            pm = [None] * CSH
            # ext2[c] = [chunk row | max] per batch entry
            ext2 = [
                sp.tile([B, 2], F32, name=f"ext{c}") for c in range(CSH)
            ]
            oh_all = [None] * len(UNITS)

            def emit_channel_head(c):
                """stream DMA + chunked max reduce + pooled column."""
                gx = gxp.tile([B, HW], F32, name=f"gx{c}", tag="gx")
                pm[c] = pmp.tile([B, NCHUNK], F32, name=f"pm{c}", tag="pm")
                if c == CSH - 1:
                    # split halves: the last reduce ends sooner after the
                    # stream ends
                    hw2, nk2 = HW // 2, NCHUNK // 2
                    for hf in range(2):
                        nc.sync.dma_start(
                            out=gx[:, hf * hw2:(hf + 1) * hw2],
                            in_=x[:, c, hf * hw2:(hf + 1) * hw2],
                        )
                        nc.vector.tensor_reduce(
                            out=pm[c][:, hf * nk2:(hf + 1) * nk2],
                            in_=gx[:, hf * hw2:(hf + 1) * hw2].rearrange(
                                "p (k j) -> p k j", k=nk2),
                            axis=mybir.AxisListType.X,
                            op=mybir.AluOpType.max,
                        )
                else:
                    nc.sync.dma_start(out=gx[:], in_=x[:, c, :])
                    nc.vector.tensor_reduce(
                        out=pm[c][:],
                        in_=gx[:].rearrange("p (k j) -> p k j", k=NCHUNK),
                        axis=mybir.AxisListType.X,
                        op=mybir.AluOpType.max,
                    )
                nc.vector.tensor_reduce(
                    out=pooled[:, c:c + 1], in_=pm[c][:],
                    axis=mybir.AxisListType.X, op=mybir.AluOpType.max,
                )
                pbc8 = sp.tile([B, 8], F32, name=f"pbc8_{c}")
                glue = nc.vector if c == CSH - 1 else nc.gpsimd
                glue.tensor_copy(
                    out=pbc8[:], in_=pooled[:, c:c + 1].to_broadcast([B, 8])
                )
                return pbc8

            def emit_channel_tail(c, pbc8):
                """argmax chunk of each map -> ext2 = [chunk row | max]."""
                cm8 = sp.tile([B, 8], U32, name=f"cm8_{c}")
                nc.vector.max_index(out=cm8[:], in_max=pbc8[:],
                                    in_values=pm[c][:])
                cmf = sp.tile([B, 1], F32, name=f"cmf{c}")
                nc.gpsimd.tensor_copy(out=cmf[:], in_=cm8[:, 0:1])
                g1 = sp.tile([B, 1], F32, name=f"g1_{c}")
                nc.gpsimd.tensor_scalar(
                    out=g1[:], in0=cmf[:], scalar1=1.0,
                    scalar2=brow[:, 0:1],
                    op0=mybir.AluOpType.mult, op1=mybir.AluOpType.add,
                )
                nc.gpsimd.tensor_scalar(
                    out=ext2[c][:, 0:1], in0=g1[:], scalar1=1.0,
                    scalar2=float(c * NCHUNK),
                    op0=mybir.AluOpType.mult, op1=mybir.AluOpType.add,
                )
                nc.gpsimd.tensor_copy(
                    out=ext2[c][:, 1:2], in_=pooled[:, c:c + 1]
                )

            def emit_topk(u):
                """transpose + top-8 + one-hot survivor columns."""
                c_lo, c_hi = UNITS[u]
                ncha = c_hi - c_lo
                pooled_t_ps = pp.tile([ncha, B], F32, name=f"ptps{u}",
                                      tag="pt")
                nc.tensor.transpose(
                    out=pooled_t_ps[:], in_=pooled[:, c_lo:c_hi],
                    identity=ident[:],
                )
                pooled_t = sp.tile([ncha, B], F32, name=f"pt{u}")
                nc.scalar.copy(out=pooled_t[:], in_=pooled_t_ps[:])
                pt8 = sp.tile([ncha, 8], F32, name=f"pt8{u}")
                nc.vector.max(out=pt8[:], in_=pooled_t[:])
                pi8 = sp.tile([ncha, 8], U32, name=f"pi8{u}")
                nc.vector.max_index(out=pi8[:], in_max=pt8[:],
                                    in_values=pooled_t[:])
                fast = u == len(UNITS) - 1   # Vector is idle post-stream
                pi8f = sp.tile([ncha, 8], F32, name=f"pi8f{u}")
                (nc.vector if fast else nc.gpsimd).tensor_copy(
                    out=pi8f[:], in_=pi8[:])
                # ohbc[b, ci*K+j] = pi8f[ci, j]: per-channel matmul with a
                # stride-0-broadcast identity column as the stationary input
                ohbc = pp.tile([B, ncha * K], F32, name=f"ohbc{u}",
                               tag="ohbc")
                for ci in range(ncha):
                    nc.tensor.matmul(
                        out=ohbc[:, ci * K:(ci + 1) * K],
                        lhsT=ident[0:ncha, ci:ci + 1].to_broadcast([ncha, B]),
                        rhs=pi8f[:, 0:K], start=True, stop=True,
                    )
                oh_all[u] = sp.tile([B, ncha * K], F32, name=f"oha{u}")
                if fast:
                    nc.vector.tensor_tensor(
                        out=oh_all[u][:], in0=ohbc[:],
                        in1=iota_b[:, 0:1].to_broadcast([B, ncha * K]),
                        op=mybir.AluOpType.is_equal,
                    )
                else:
                    s1 = sp.tile([B, ncha * K], F32, name=f"s1_{u}")
                    nc.scalar.activation(
                        out=s1[:], in_=ohbc[:],
                        func=mybir.ActivationFunctionType.Identity,
                        bias=iota_b[:, 0:1], scale=-1.0,
                    )
                    s2 = sp.tile([B, ncha * K], F32, name=f"s2_{u}")
                    nc.scalar.activation(
                        out=s2[:], in_=s1[:],
                        func=mybir.ActivationFunctionType.Square,
                    )
                    nc.scalar.activation(
                        out=oh_all[u][:], in_=s2[:],
                        func=mybir.ActivationFunctionType.Relu,
                        bias=1.0, scale=-1.0,
                    )

            def emit_merge(u):
                """compact survivors, gather chunks, mask, scatter."""
                c_lo, c_hi = UNITS[u]
                ncha = c_hi - c_lo
                nsurv = ncha * K
                cpsT = pp.tile([2, nsurv], F32, name=f"cpsT{u}", tag="cpsT")
                for ci in range(ncha):
                    c = c_lo + ci
                    nc.tensor.matmul(
                        out=cpsT[:, ci * K:(ci + 1) * K], lhsT=ext2[c][:],
                        rhs=oh_all[u][:, ci * K:(ci + 1) * K],
                        start=True, stop=True,
                    )
                fast = u == len(UNITS) - 1
                cpsT_sb = sp.tile([2, nsurv], F32, name=f"cpsTs{u}")
                (nc.vector if fast else nc.scalar).tensor_copy(
                    out=cpsT_sb[:], in_=cpsT[:]
                ) if fast else nc.scalar.copy(out=cpsT_sb[:], in_=cpsT[:])
                cps = pp.tile([nsurv, 2], F32, name=f"cps{u}", tag="cps")
                nc.tensor.transpose(
                    out=cps[:], in_=cpsT_sb[:], identity=ident[0:2, 0:2]
                )
                ce = sp.tile([nsurv, 2], F32, name=f"ce{u}")
                if fast:
                    nc.vector.tensor_copy(out=ce[:], in_=cps[:])
                else:
                    nc.scalar.copy(out=ce[:], in_=cps[:])
                rows_i = sp.tile([nsurv, 1], I32, name=f"rows{u}")
                (nc.vector if fast else nc.gpsimd).tensor_copy(
                    out=rows_i[:], in_=ce[:, 0:1])
                # gather the surviving chunks
                cx = sp.tile([nsurv, CHW], F32, name=f"cx{u}")
                nc.gpsimd.indirect_dma_start(
                    out=cx[:], out_offset=None, in_=x_rows[:],
                    in_offset=bass.IndirectOffsetOnAxis(
                        ap=rows_i[:, 0:1], axis=0
                    ),
                )
                # mask each chunk to {M at positions == M, 0 elsewhere}
                mkv = sp.tile([nsurv, CHW], F32, name=f"mkv{u}")
                if fast:
                    meq = sp.tile([nsurv, CHW], F32, name=f"meq{u}")
                    nc.vector.tensor_tensor(
                        out=meq[:], in0=cx[:],
                        in1=ce[:, 1:2].to_broadcast([nsurv, CHW]),
                        op=mybir.AluOpType.is_equal,
                    )
                    nc.vector.tensor_tensor(
                        out=mkv[:], in0=meq[:],
                        in1=ce[:, 1:2].to_broadcast([nsurv, CHW]),
                        op=mybir.AluOpType.mult,
                    )
                else:
                    s0 = sp.tile([nsurv, CHW], F32, name=f"ms0_{u}")
                    nc.scalar.activation(
                        out=s0[:], in_=cx[:],
                        func=mybir.ActivationFunctionType.Identity,
                        bias=ce[:, 1:2], scale=-1.0,
                    )
                    ind = sp.tile([nsurv, CHW], F32, name=f"mind{u}")
                    nc.scalar.activation(
                        out=ind[:], in_=s0[:],
                        func=mybir.ActivationFunctionType.Relu,
                        bias=1.0, scale=-float(1 << 30),
                    )
                    nc.scalar.activation(
                        out=mkv[:], in_=ind[:],
                        func=mybir.ActivationFunctionType.Identity,
                        bias=0.0, scale=ce[:, 1:2],
                    )
                nc.gpsimd.indirect_dma_start(
                    out=y_rows[:],
                    out_offset=bass.IndirectOffsetOnAxis(
                        ap=rows_i[:, 0:1], axis=0
                    ),
                    in_=mkv[:], in_offset=None,
                )

            # ---- emission: stream with per-channel tails one behind, unit
            # ---- top-k + merge one channel after each unit boundary.
            pbc = [None] * CSH
            for c in range(CSH):
                pbc[c] = emit_channel_head(c)
                if c >= 1:
                    emit_channel_tail(c - 1, pbc[c - 1])
                if c in (5, 9, 13):
                    u = c // 4 - 1
                    emit_topk(u)
                    emit_merge(u)
            emit_channel_tail(CSH - 1, pbc[CSH - 1])
            emit_topk(3)
            emit_merge(3)

    nc.finalize()
    _nc_cache = nc
    return nc


def _install_profile_hook():
    """Inject the antenv.axon_hooks shim so trace=True captures NTFFs."""
    import sys
    import types

    if "antenv.axon_hooks" in sys.modules:
        return
    import antenv
    import trn_agent_boot.trn_boot as tb

    mod = types.ModuleType("antenv.axon_hooks")
    mod._hook = tb._ntff_profile_via_ctypes("/opt/axon/libaxon_pjrt.so")
    mod.get_axon_ntff_profile_hook = lambda: mod._hook
    mod.set_axon_ntff_profile_hook = lambda h: setattr(mod, "_hook", h)
    sys.modules["antenv.axon_hooks"] = mod
    antenv.axon_hooks = mod

    # no S3 in this container — keep artifacts local
    import concourse.bass_utils as bu

    bu.upload_artifacts = lambda tmpdir: tmpdir


def run(activations, trace=False):
    if trace:
        _install_profile_hook()
    act = np.asarray(activations)
    assert act.shape == (B, C_FULL, H, W), act.shape
    act = act.astype(np.float32, copy=False)
    nc = _build()
    in_maps = [
        {"x": np.ascontiguousarray(act[:, i * CSH:(i + 1) * CSH]).reshape(B, CSH, HW)}
        for i in range(N_CORES)
    ]
    res = run_bass_kernel_spmd(
        nc, in_maps, core_ids=list(range(N_CORES)), trace=trace
    )
    out = np.concatenate(
        [r["y"].reshape(B, CSH, H, W) for r in res.results], axis=1
    )
    return out, res


def kernel(activations):
    out, _ = run(activations, trace=False)
    return out

